# revision 30
# baseline (speedup 1.0000x reference)
"""MoE layer (top-2 of 8 experts, SwiGLU) on 8 Trainium2 NeuronCores.

Expert-parallel (per the sharding hint): the host computes the router
(gate logits -> top-2 -> softmax) in fp32, gathers each expert's tokens
(the "all-to-all dispatch"), each core runs a dense SwiGLU MLP over one
expert's tokens in bf16 (fp32 PSUM accumulation), and the host applies
the combine weights and scatter-adds back to token order.

Capacity-1.0 expert parallelism: each core computes at most
ceil(N*K/8) = 2048 of its expert's pairs; overflow pairs (~1% under
near-balanced routing) are computed exactly on the host in fp32. This
removes the padding of every core to the max expert count (2137 for
the reference routing) — PE work per core drops to the algebraic
floor of 2048 tokens x 768 PE-rows/token. (Measured sustained PE
clock is ~1.92-2.05 GHz, not the nominal 2.4; the kernel is PE-row
bound at ~98% occupancy, so row count is the only real lever. fp8
DoubleRow would halve rows but costs >=3% rel err vs the 2% budget,
dead; dropping low-weight pairs costs 2.4e-2 at 0.5% dropped, dead.)

Kernel structure (per core, C = min(max expert count, 2048)):
  - stage-1 (h = silu(x@wg.T) * (x@w1.T)) in 512-token blocks; w1 AND
    wg resident in SBUF (weight-stream DMA measurably taxes the PE;
    working-set pools shrink to fit 128 KB/partition of weights — a
    tailed C falls back to streaming wg); weights stationary, tokens
    moving (N=512 streams keep LDWEIGHTS hidden). Block-0 loads are
    split into small pieces issued across SP+Act sequencers (DMA
    issue is ~0.6us each per sequencer) so the first matmul starts
    ~3.4us in.
  - stage-2 (y = h@w2.T) streams w2 two 128x512 chunks per DMA; if a
    tail block exists (C not a multiple of 512) its stage-2 merges
    into the last full block's pass.
  - y returned in bf16 (host upcasts; quantization well inside budget).
PSUM: psg 2 + ps1t 1 + psy 4-5 = 7-8 banks.
"""

import numpy as np
import ml_dtypes

import concourse.bass as bass
import concourse.mybir as mybir
import concourse.tile as tile
from concourse.bass_utils import run_bass_kernel_spmd

# ---------------------------------------------------------------------------
# Workaround for this walrus build: TPB instructions have a single hardware
# wait slot; split k-wait instructions into k-1 single-wait NOPs + the
# original (program-order semantics identical).
# ---------------------------------------------------------------------------

_ws_counter = [0]


def _split_multi_waits(nc: bass.Bass) -> int:
    n_split = 0
    for f in nc.m.functions:
        for bb in f.blocks:
            new_insts = []
            for inst in bb.instructions:
                si = inst.sync_info
                if si is not None and si.on_wait and len(si.on_wait) > 1:
                    waits = list(si.on_wait)
                    for w in waits[:-1]:
                        _ws_counter[0] += 1
                        n_split += 1
                        new_insts.append(
                            mybir.InstNoOp(
                                name=f"waitsplit-{_ws_counter[0]}",
                                opcode="NoOp",
                                engine=inst.engine,
                                sync_info=mybir.SyncInfo(
                                    on_wait=[w], on_update=[]
                                ),
                                bass_nofuse=True,
                                text_hint="waitsplit",
                            )
                        )
                    si.on_wait = [waits[-1]]
                new_insts.append(inst)
            bb.instructions[:] = new_insts
    return n_split

# ---------------------------------------------------------------------------

D = 1024
DFF = 4096
N_EXPERTS = 8
TOP_K = 2
N_CORES = 8
TB = 512          # full token block
WG_BUFS = 5
XT_BUFS = 3
W2_BUFS = 8       # bufs of 2-chunk w2 tiles
H_BUFS = 36
PSG_BUFS = 2
PS1_BUFS = 1
PSY_BUFS = 5
KD = D // 128     # 8 contraction tiles over d
NF = DFF // 128   # 32 tiles over d_ff

BF16 = mybir.dt.bfloat16
F32 = mybir.dt.float32
NP_BF16 = ml_dtypes.bfloat16

_NC_CACHE: dict[int, bass.Bass] = {}


def _build_kernel(C: int, repeat: int = 1, unroll: int = 1) -> bass.Bass:
    """Dense SwiGLU MLP over C tokens (C a multiple of 32).

    Blocks: full 512-token blocks, then one tail of C%512 (if any). The
    tail's stage-2 is folded into the last full block's stage-2 pass.

    repeat>1 wraps the computation in a hardware For_i loop for
    wall-clock slope calibration (resident w1 loads once, outside).
    unroll>1 repeats the body in Python instead (for TimelineSim, which
    cannot resolve For_i register branches)."""
    assert C >= 128
    tail = C % TB
    n_full = C // TB
    # Tail last: during the earlier blocks' stage-2 passes the wg stream
    # runs ahead, so the tail's fast-consuming stage-1 isn't DMA-paced.
    blocks = [TB] * n_full + ([tail] if tail else [])
    # stage-2 groups: indices of blocks whose stage-2 runs as one pass;
    # the tail's stage-2 is merged with the last full block's when the
    # combined m-tile count fits the PSY_BUFS psum banks.
    groups = [[i] for i in range(len(blocks))]
    merged = bool(
        tail and n_full and TB // 128 + (tail + 127) // 128 <= PSY_BUFS
    )
    if merged:
        groups = groups[:-2] + [[len(blocks) - 2, len(blocks) - 1]]

    # wg residency: weight-stream DMA measurably taxes the PE (~3.4%
    # for wg+w2 streams, interleaved A/B), so keep wg resident like w1
    # when SBUF allows (no tail block; the tailed fallback streams wg
    # as before). Working-set pools shrink to make the 64 KB/partition
    # of wg residency fit.
    wg_resident = tail == 0
    xt_bufs = 2 if wg_resident else XT_BUFS
    h_bufs = 34 if wg_resident else H_BUFS
    w2_bufs = 6 if wg_resident else W2_BUFS
    hg_bufs = 2 if wg_resident else 3

    nc = bass.Bass()
    xt = nc.dram_tensor("xt", [128, KD, C], BF16, kind="ExternalInput")
    w1t = nc.dram_tensor("w1t", [128, KD, DFF], BF16, kind="ExternalInput")
    wgt = nc.dram_tensor("wgt", [128, KD, DFF], BF16, kind="ExternalInput")
    w2t = nc.dram_tensor("w2t", [128, NF, D], BF16, kind="ExternalInput")
    y = nc.dram_tensor("y", [C, D], BF16, kind="ExternalOutput")

    silu = mybir.ActivationFunctionType.Silu

    with tile.TileContext(nc) as tc:
        with (
            tc.tile_pool(name="wres", bufs=1) as wres,
            tc.tile_pool(name="wg", bufs=1 if wg_resident else WG_BUFS)
            as wgpool,
            tc.tile_pool(name="xt", bufs=xt_bufs) as xtpool,
            tc.tile_pool(name="hg", bufs=hg_bufs) as hgpool,
            tc.tile_pool(name="h", bufs=h_bufs) as hpool,
            tc.tile_pool(name="ht", bufs=NF) as htpool,
            tc.tile_pool(name="w2", bufs=w2_bufs) as w2pool,
            tc.tile_pool(name="yo", bufs=4) as ypool,
            tc.tile_pool(name="ps1", bufs=1, space="PSUM") as psum1,
            tc.tile_pool(name="ps2", bufs=PSY_BUFS, space="PSUM") as psum2,
        ):
            # Resident w1, split into 8 dff-chunks so the first matmuls only
            # wait on the chunk they need (loaded just-in-time in block 0).
            # Chunk 0 is further split into four 128-col tiles so its loads
            # interleave with the wg prologue pieces on the SP queue and
            # the first ps1t matmul waits on 256 KB, not 1 MB.
            w1p0_cols = [
                wres.tile([128, KD, 128], BF16, tag=f"w1p0c{j}",
                          name=f"w1p0c{j}")
                for j in range(4)
            ]
            w1_parts = [None] + [
                wres.tile([128, KD, 512], BF16, tag=f"w1p{i}", name=f"w1p{i}")
                for i in range(1, NF // 4)
            ]
            # wg chunk-0 storage: two contraction-half pieces for col 0
            # (so the very first Ldweights waits on 128 KB) + three
            # 128-col pieces. Resident mode adds chunks 1..7.
            wg00 = [
                wgpool.tile([128, KD // 2, 128], BF16, bufs=1,
                            tag=f"wg00{h}", name=f"wg00{h}")
                for h in range(2)
            ]
            wg0_pieces = [None] + [
                wgpool.tile([128, KD, 128], BF16, bufs=1,
                            tag=f"wg0p{i}", name=f"wg0p{i}")
                for i in range(1, 4)
            ]
            wg_parts = [None] + ([
                wgpool.tile([128, KD, 512], BF16, bufs=1,
                            tag=f"wgp{i}", name=f"wgp{i}")
                for i in range(1, NF // 4)
            ] if wg_resident else [None] * (NF // 4 - 1))

            if repeat > 1 or unroll > 1:
                # calibration mode: load resident weights once, outside
                # the loop
                for h in range(2):
                    nc.sync.dma_start(
                        wg00[h][:],
                        wgt[:, h * (KD // 2):(h + 1) * (KD // 2), 0:128],
                    )
                for i in range(1, 4):
                    nc.sync.dma_start(
                        wg0_pieces[i][:], wgt[:, :, i * 128:(i + 1) * 128]
                    )
                for j in range(4):
                    nc.sync.dma_start(
                        w1p0_cols[j][:], w1t[:, :, j * 128:(j + 1) * 128]
                    )
                for i in range(1, NF // 4):
                    if wg_resident:
                        nc.sync.dma_start(
                            wg_parts[i][:], wgt[:, :, i * 512:(i + 1) * 512]
                        )
                    nc.sync.dma_start(
                        w1_parts[i][:], w1t[:, :, i * 512:(i + 1) * 512]
                    )

            def _stage1(b, tb, tok0, h_tiles):
                """SwiGLU hidden for tokens [tok0, tok0+tb); appends the 32
                [128, tb] bf16 h tiles to h_tiles."""
                # A merged tail's h tiles live concurrently with the last
                # full block's, so they need their own (small: tail<=128)
                # pool; an unmerged tail's stage-2 follows immediately, so
                # the main pool's lifetimes work and a [128, tail<=511]
                # 32-buf side pool would overflow SBUF.
                use_ht = tb != TB and merged
                pool = htpool if use_ht else hpool
                jit = repeat == 1 and unroll == 1
                if b == 0:
                    # Prologue (JIT mode): the first matmul needs wg
                    # piece 0 + xt d-tile 0 only. DMA issue serializes
                    # per sequencer (~0.6 us each), so issue critical
                    # pieces first, interleaved with the w1-chunk-0 col
                    # loads in the order the j-loop consumes them, and
                    # put the xt d-tiles on the Act sequencer.
                    if jit:
                        for h in range(2):
                            nc.sync.dma_start(
                                wg00[h][:],
                                wgt[:, h * (KD // 2):(h + 1) * (KD // 2),
                                    0:128],
                            )
                        nc.sync.dma_start(
                            w1p0_cols[0][:], w1t[:, :, 0:128]
                        )
                        for i in range(1, 4):
                            nc.sync.dma_start(
                                wg0_pieces[i][:],
                                wgt[:, :, i * 128:(i + 1) * 128],
                            )
                            nc.sync.dma_start(
                                w1p0_cols[i][:],
                                w1t[:, :, i * 128:(i + 1) * 128],
                            )
                    xt_parts = [
                        xtpool.tile([128, 2, tb], BF16, bufs=1,
                                    tag=f"xt0d{d}", name=f"xt0d{d}")
                        for d in range(KD // 2)
                    ]
                    for d in range(KD // 2):
                        nc.scalar.dma_start(
                            xt_parts[d][:],
                            xt[:, 2 * d:2 * d + 2, tok0:tok0 + tb],
                        )
                    xslice = (  # noqa: E731
                        lambda d: xt_parts[d // 2][:, d % 2, :]
                    )
                else:
                    xt_sb = xtpool.tile([128, KD, tb], BF16, tag="xt")
                    nc.sync.dma_start(xt_sb[:], xt[:, :, tok0:tok0 + tb])
                    xslice = lambda d: xt_sb[:, d, :]  # noqa: E731
                for dfc in range(NF // 4):
                    if dfc == 0:
                        # chunk-0 pieces are persistent; all blocks use
                        # them
                        wg_pieces = wg0_pieces
                        wg_ch = None
                    elif wg_resident:
                        wg_pieces = None
                        wg_ch = wg_parts[dfc]
                        if b == 0 and jit:
                            nc.sync.dma_start(
                                wg_ch[:],
                                wgt[:, :, dfc * 512:(dfc + 1) * 512],
                            )
                    else:
                        wg_pieces = None
                        wg_ch = wgpool.tile([128, KD, 512], BF16, tag="wg")
                        nc.sync.dma_start(
                            wg_ch[:], wgt[:, :, dfc * 512:(dfc + 1) * 512]
                        )
                    if b == 0 and dfc > 0 and jit:
                        nc.sync.dma_start(
                            w1_parts[dfc][:],
                            w1t[:, :, dfc * 512:(dfc + 1) * 512],
                        )
                    for j in range(4):
                        psg = psum1.tile([128, tb], F32, tag="psg",
                                         bufs=PSG_BUFS)
                        for d in range(KD):
                            if wg_pieces is not None:
                                if j == 0:
                                    wslice = wg00[d // (KD // 2)][
                                        :, d % (KD // 2), :
                                    ]
                                else:
                                    wslice = wg_pieces[j][:, d, :]
                            else:
                                wslice = wg_ch[:, d, j * 128:(j + 1) * 128]
                            nc.tensor.matmul(
                                psg[:],
                                wslice,
                                xslice(d),
                                start=(d == 0),
                                stop=(d == KD - 1),
                            )
                        ps1t = psum1.tile([128, tb], F32, tag="ps1t",
                                          bufs=PS1_BUFS)
                        for d in range(KD):
                            if dfc == 0:
                                w1slice = w1p0_cols[j][:, d, :]
                            else:
                                w1slice = w1_parts[dfc][
                                    :, d, j * 128:(j + 1) * 128
                                ]
                            nc.tensor.matmul(
                                ps1t[:],
                                w1slice,
                                xslice(d),
                                start=(d == 0),
                                stop=(d == KD - 1),
                            )
                        hg = hgpool.tile([128, tb], BF16, tag="hg")
                        nc.scalar.activation(hg[:], psg[:], silu)
                        h = pool.tile([128, tb], BF16,
                                      tag="ht" if use_ht else "h")
                        nc.vector.tensor_mul(h[:], hg[:], ps1t[:])
                        h_tiles.append(h)

            def _stage2(mtiles):
                """One stage-2 pass: mtiles = list of (h_tiles, m, mt, ytok)
                with mt tokens each; every w2 chunk is used by all mtiles."""
                for half in range(2):
                    psys = [
                        psum2.tile([128, 512], F32, tag="psy", name=f"psy{i}")
                        for i in range(len(mtiles))
                    ]
                    for dfp in range(NF // 2):
                        w2_ch = w2pool.tile([128, 2, 512], BF16, tag="w2c")
                        nc.sync.dma_start(
                            w2_ch[:],
                            w2t[:, 2 * dfp:2 * dfp + 2,
                                half * 512:(half + 1) * 512],
                        )
                        for k in range(2):
                            df = 2 * dfp + k
                            for i, (ht, m, mt, _) in enumerate(mtiles):
                                nc.tensor.matmul(
                                    psys[i][:mt, :],
                                    ht[df][:, m * 128:m * 128 + mt],
                                    w2_ch[:, k, :],
                                    start=(df == 0),
                                    stop=(df == NF - 1),
                                )
                    # Drain split across DVE and Act (+ SP/Act DMA issue)
                    # so the final pass's writeback chain is ~half as long.
                    for i, (_, _, mt, ytok) in enumerate(mtiles):
                        y_sb = ypool.tile([128, 512], BF16, tag="ysb")
                        if i % 2 == 0:
                            nc.vector.tensor_copy(
                                y_sb[:mt, :], psys[i][:mt, :]
                            )
                            nc.sync.dma_start(
                                y[ytok:ytok + mt,
                                  half * 512:(half + 1) * 512],
                                y_sb[:mt, :],
                            )
                        else:
                            nc.scalar.activation(
                                y_sb[:mt, :], psys[i][:mt, :],
                                mybir.ActivationFunctionType.Copy,
                            )
                            nc.scalar.dma_start(
                                y[ytok:ytok + mt,
                                  half * 512:(half + 1) * 512],
                                y_sb[:mt, :],
                            )

            def _trace_body():
                tok0s = np.concatenate([[0], np.cumsum(blocks)])
                done = 0
                for g in groups:
                    per_block_h = []
                    for b in g:
                        h_tiles = []
                        _stage1(b, blocks[b], int(tok0s[b]), h_tiles)
                        per_block_h.append(h_tiles)
                    mtiles = []
                    for bi, b in enumerate(g):
                        tb, t0 = blocks[b], int(tok0s[b])
                        for m in range((tb + 127) // 128):
                            mt = min(128, tb - m * 128)
                            mtiles.append(
                                (per_block_h[bi], m, mt, t0 + m * 128)
                            )
                    _stage2(mtiles)
                    done += len(g)

            if repeat == 1:
                for _ in range(unroll):
                    _trace_body()
            else:
                with tc.For_i(0, repeat, 1):
                    _trace_body()
    _split_multi_waits(nc)
    return nc


def _swizzle_k(a: np.ndarray) -> np.ndarray:
    """[K, F] -> [128, K//128, F] with K = ko*128 + p on partitions."""
    k, f = a.shape
    return np.ascontiguousarray(
        a.reshape(k // 128, 128, f).transpose(1, 0, 2)
    )


def kernel(x, gate_w, w1, w_gate, w2):
    x = np.asarray(x)
    gate_w = np.asarray(gate_w)
    w1, w_gate, w2 = np.asarray(w1), np.asarray(w_gate), np.asarray(w2)
    b, t, d = x.shape
    xf = np.ascontiguousarray(x.reshape(-1, d)).astype(np.float32)
    n_tok = xf.shape[0]

    # --- Router (host, fp32, mirrors reference math) ---
    logits = xf @ gate_w.T.astype(np.float32)                  # [N, E]
    top_idx = np.argsort(-logits, axis=1, kind="stable")[:, :TOP_K]  # [N, K]
    top_vals = np.take_along_axis(logits, top_idx, axis=1)
    m = top_vals.max(axis=1, keepdims=True)
    ex = np.exp(top_vals - m)
    top_w = ex / ex.sum(axis=1, keepdims=True)                 # [N, K]

    pair_expert = top_idx.reshape(-1)                          # [N*K]
    pair_w = top_w.reshape(-1).astype(np.float32)
    order = np.argsort(pair_expert, kind="stable")
    counts = np.bincount(pair_expert, minlength=N_EXPERTS)
    starts = np.concatenate([[0], np.cumsum(counts)])

    # Capacity-1.0 expert parallelism: each core computes at most one
    # balanced share (ceil(N*K / n_cores)) of its expert's pairs; the
    # overflow (~1% of pairs under near-balanced routing) is computed
    # exactly on the host in fp32. This removes the padding of every
    # core to the max expert count, the dominant PE-work imbalance.
    cap = -(-n_tok * TOP_K // N_CORES)
    C = max(128, min(int(counts.max()), cap))

    # --- Build per-core inputs (dispatch) ---
    in_maps = []
    sels = []
    host_sels = []
    for e in range(N_EXPERTS):
        sel_all = order[starts[e]:starts[e + 1]]
        sel = sel_all[:C]
        sels.append(sel)
        host_sels.append(sel_all[C:])
        tok = sel // TOP_K
        xt_full = np.zeros((D, C), dtype=np.float32)
        xt_full[:, : len(tok)] = xf[tok].T
        in_maps.append(
            {
                "xt": _swizzle_k(xt_full).astype(NP_BF16),
                "w1t": _swizzle_k(
                    np.ascontiguousarray(w1[e].T).astype(np.float32)
                ).astype(NP_BF16),
                "wgt": _swizzle_k(
                    np.ascontiguousarray(w_gate[e].T).astype(np.float32)
                ).astype(NP_BF16),
                "w2t": _swizzle_k(
                    np.ascontiguousarray(w2[e].T).astype(np.float32)
                ).astype(NP_BF16),
            }
        )

    if C not in _NC_CACHE:
        _NC_CACHE[C] = _build_kernel(C)
    nc = _NC_CACHE[C]

    res = run_bass_kernel_spmd(nc, in_maps, core_ids=list(range(N_CORES)))

    # --- Combine (host): weight by router prob, scatter-add to tokens ---
    contrib = np.zeros((n_tok * TOP_K, D), dtype=np.float32)
    for e in range(N_EXPERTS):
        sel = sels[e]
        y_e = res.results[e]["y"][: len(sel)].astype(np.float32)
        contrib[sel] = y_e * pair_w[sel][:, None]

    # --- Capacity-overflow pairs: exact fp32 on host ---
    for e in range(N_EXPERTS):
        hsel = host_sels[e]
        if len(hsel) == 0:
            continue
        xe = xf[hsel // TOP_K]                                 # [m, D]
        g = xe @ w_gate[e].T.astype(np.float32)
        a = xe @ w1[e].T.astype(np.float32)
        h = (g / (1.0 + np.exp(-g))) * a
        y_e = h @ w2[e].T.astype(np.float32)
        contrib[hsel] = y_e * pair_w[hsel][:, None]

    out = contrib.reshape(n_tok, TOP_K, D).sum(axis=1)
    return out.reshape(b, t, d).astype(x.dtype)

